# revision 31
# baseline (speedup 1.0000x reference)
"""DigitCapsules dynamic-routing kernel for 8 Trainium2 NeuronCores.

Problem: x [64, 2048, 8] f32, W [1, 2048, 32, 16, 8] f32 ->
  u_hat[b,i,j,o] = sum_d W[0,i,j,o,d] * x[b,i,d]
  3 routing iterations (softmax over j=32 caps, weighted sum over i=2048,
  squash over o=16, agreement update), output v [64, 32, 16].

Strategy: shard in_caps (i) across the 8 cores (256 i's each). u_hat is
IDENTICAL across routing iterations, so it is produced ONCE (PE block-diag
matmuls + fp16 PSUM->SBUF copies) and kept resident in SBUF for the whole
kernel: 8 batch-groups x [128, 8192] f16 = 128KB/partition. The w and bdx
input arenas are recycled into the last two u16 tiles in an order that lets
the copies interleave with the matmuls still reading them (bdx is bg-major
so bg7's copies land in the long-consumed bg0-3 region; bg6 overwrites w in
the same pr order its own matmuls read it). Production of bgs 4-7 is
interleaved into pass B so the PE queue reaches pass-B matmuls early.

Passes B and C are then pure vector work per bg:
  uv = u16 * v (one 8192-elem DVE op), o-tree reduce IN PLACE inside uv
  (DVE l1/l2, GpSimd l3/l4), exp (ScalarE), cu = exp(b) * u16 (DVE+GpSimd),
  s = PSUM matmuls whose stationary is a per-i-tile 1/esum block-diagonal
  (softmax denominator folded into the matmul - no separate normalize op).
Cross-core traffic: AllReduce of s partials, chunked so the last AR covers
only 8 batches (short exit tail). v is replicated across partitions by PE
matmuls with a block-diag identity stationary (no DMA storms).

Device layouts (per core):
  K-partitions (i16, d): k = i16*8 + d       (16 i's x 8 in_dims = 128)
  M-partitions (ip, b8): p = ip*8 + b8       (16 i's x 8 batch = 128)
  w    [128, 16*512]  : [(i16,d), (it, o, j)]          -- W slice (arena)
  xs   [128, 16*64]   : [(i16,d), (it, b)]             -- x slice (pass-A lhsT)
  bdx  [128, 8*16*128]: [(i16,d), (bg, it, ip, b8)]    -- block-diag x (arena)
  u16[bg] [128, 16*512]: [(ip,b8), (it, o, j)]         -- resident u_hat f16
"""
import sys

sys.path.insert(0, "/opt/trn_rl_repo")

import numpy as np
import concourse.bass as bass
import concourse.mybir as mybir
import concourse.tile as tile
from concourse.vector_clock import ScopedClock
from concourse.bass_utils import run_bass_kernel_spmd

# ---------------------------------------------------------------------------
# Workaround: this walrus build rejects semaphore waits attached to InstDrain
# ("Too many sync wait commands", CoreV3GenImpl setupSyncWait NO_STRUCT) and
# allows at most one wait per instruction. Emit bare drains + sequencer-level
# barriers, and hoist the Tile tail-drain waits onto single-wait NOPs.
# ---------------------------------------------------------------------------


def _safe_multi_engine_barrier(self, engines):
    for eng_type in engines:
        d = mybir.InstDrain(
            name=self.get_next_instruction_name(),
            ins=[],
            outs=[],
            bass_is_fusable=False,
        )
        d.engine = eng_type
        self.engines[eng_type].add_instruction(d)
    for inst in self._sem_only_all_engine_barrier_insts(f"aeb{self.next_id()}"):
        self.engines[inst.engine].add_instruction(inst)


def _safe_drain_and_barrier(self, tick_clock, wait_clock):
    nop_inst = self.nc.sync.nop(nofuse=True)
    wait_clock.add_sem_waits(nop_inst.ins, ScopedClock({None: tick_clock.global_clock}))
    waits = list(nop_inst.ins.sync_info.on_wait or [])
    if len(waits) > 1:
        si = nop_inst.ins.sync_info
        si.on_wait = waits[:1]
        nop_inst.ins.sync_info = si
        for w in waits[1:]:
            extra = self.nc.sync.nop(nofuse=True)
            extra.ins.sync_info = mybir.SyncInfo(on_wait=[w], on_update=[])
    self.nc.sync.drain()
    self.nc.all_engine_barrier()
    assert self.sems is not None
    popped = self.nc._tile_sem_poison_stack.pop()
    assert popped is self._sem_poison
    self.nc.clear_and_free_semaphores(list(self.sems.allocated().values()))
    self.nc.all_engine_barrier()


bass.Bass.multi_engine_barrier = _safe_multi_engine_barrier
tile.TileContext._drain_and_barrier = _safe_drain_and_barrier


def _split_multi_waits(nc):
    """This walrus encodes at most ONE semaphore wait per instruction (zero
    on InstDrain). Hoist excess waits onto single-wait NOPs inserted just
    before the instruction on the same engine — identical semantics, since
    each engine executes its block subsequence in order."""
    uid = 0
    for f in nc.m.functions:
        for blk in f.blocks:
            out = []
            changed = False
            for inst in blk.instructions:
                si = getattr(inst, "sync_info", None)
                waits = list(si.on_wait) if si is not None and si.on_wait else []
                limit = 0 if isinstance(inst, mybir.InstDrain) else 1
                if len(waits) > limit:
                    for w in waits[: len(waits) - limit]:
                        nop = mybir.InstNoOp(
                            name=f"{inst.name}-wsplit{uid}", ins=[], outs=[])
                        uid += 1
                        nop.engine = inst.engine
                        nop.sync_info = mybir.SyncInfo(on_wait=[w], on_update=[])
                        out.append(nop)
                    inst.sync_info = mybir.SyncInfo(
                        on_wait=waits[len(waits) - limit:],
                        on_update=list(si.on_update or []),
                    )
                    changed = True
                out.append(inst)
            if changed:
                blk.instructions = out

# ---------------------------------------------------------------------------
# Problem constants (hardcoded per contract)
# ---------------------------------------------------------------------------
B, I, J, O, D = 64, 2048, 32, 16, 8
N_CORES = 8
IL = I // N_CORES          # 256 local in_caps per core
IT = IL // 16              # 16 i-tiles of 16 i's
NBG = B // 8               # 8 batch groups of 8
JO = J * O                 # 512
EPS = 1e-8
F32 = mybir.dt.float32
F16 = mybir.dt.float16
AX = mybir.AxisListType
ALU = mybir.AluOpType
ACTF = mybir.ActivationFunctionType


def build_nc(detect_races=True):
    nc = bass.Bass(num_devices=N_CORES, detect_race_conditions=detect_races)
    w_in = nc.dram_tensor("w_in", [128, IT * JO], F16, kind="ExternalInput")
    xs_in = nc.dram_tensor("xs_in", [128, IT * B], F16, kind="ExternalInput")
    bdx_in = nc.dram_tensor("bdx_in", [128, IT * NBG * 128], F16, kind="ExternalInput")
    ones_in = nc.dram_tensor("ones_in", [128, 8], F16, kind="ExternalInput")
    rep_in = nc.dram_tensor("rep_in", [32, 4 * 128], F16, kind="ExternalInput")
    v_out = nc.dram_tensor("v_out", [B, JO], F16, kind="ExternalOutput")

    groups = [list(range(N_CORES))]
    LN_J = float(np.log(J))

    with tile.TileContext(nc) as tc:
        with (
            tc.tile_pool(name="res", bufs=1) as res,
            tc.tile_pool(name="work", bufs=1) as work,
            tc.tile_pool(name="small", bufs=2) as small,
            tc.tile_pool(name="sq", bufs=1) as sqp,
            tc.tile_pool(name="upsum", bufs=3, space="PSUM") as upsum,
            tc.tile_pool(name="spsum", bufs=1, space="PSUM") as spsum,
            tc.tile_pool(name="dram", bufs=2, space="DRAM") as dram,
        ):
            # ---- resident tiles ----
            w_sb = res.tile([128, IT * JO], F16)
            bdx_sb = res.tile([128, IT * NBG * 128], F16)
            xs_sb = res.tile([128, IT * B], F16)
            ones_sb = res.tile([128, 8], F16)
            rep_sb = res.tile([32, 4 * 128], F16)
            b_state = res.tile([128, NBG * IT * J], F16)
            vrep = res.tile([128, NBG * JO], F16)
            u16_fresh = [res.tile([128, IT * JO], F16, name=f"u16_{g}")
                         for g in range(6)]
            eps_sb = res.tile([48, 1], F32)
            lnj_sb = res.tile([48, 1], F32)
            nc.gpsimd.memset(eps_sb[:], EPS)
            nc.gpsimd.memset(lnj_sb[:], LN_J)

            def u16(bg):
                # bg7 recycles the long-consumed bg0-3 half of the bdx
                # arena; bg6 recycles the w arena (produced last).
                if bg < 6:
                    return u16_fresh[bg]
                if bg == 6:
                    return w_sb
                return bdx_sb[:, 0:IT * JO]

            ared_scratch = xs_sb[:, 0:IT * J]   # xs is free after pass A

            # ---- input loads: w + xs first (pass A), then bdx ----
            for q in range(4):
                qs = (IT * JO) // 4
                nc.sync.dma_start(out=w_sb[:, q * qs:(q + 1) * qs],
                                  in_=w_in[:, q * qs:(q + 1) * qs])
            nc.sync.dma_start(out=xs_sb[:], in_=xs_in[:])
            nc.sync.dma_start(out=ones_sb[:], in_=ones_in[:])
            nc.sync.dma_start(out=rep_sb[:], in_=rep_in[:])
            for q in range(4):
                qs = (IT * NBG * 128) // 4
                nc.sync.dma_start(out=bdx_sb[:, q * qs:(q + 1) * qs],
                                  in_=bdx_in[:, q * qs:(q + 1) * qs])

            def produce_u16(bg, engines):
                """One-time u_hat production for one bg: 16 block-diag PE
                matmuls into paired PSUM tiles + fp16 copies to SBUF.
                engines: 8-char string of copy engines per pr (S/D/G)."""
                dst = u16(bg)
                for pr in range(8):
                    up = upsum.tile([128, 2 * JO], F32, tag="u")
                    for k in range(2):
                        it = pr * 2 + k
                        nc.tensor.matmul(
                            up[:, k * JO:(k + 1) * JO],
                            lhsT=bdx_sb[:, (bg * IT + it) * 128:(bg * IT + it + 1) * 128],
                            rhs=w_sb[:, it * JO:(it + 1) * JO],
                            start=True, stop=True,
                        )
                    dslice = dst[:, pr * 2 * JO:(pr + 1) * 2 * JO]
                    # GpSimd cannot read PSUM on this hw: ScalarE/DVE only
                    if engines[pr] == "S":
                        nc.scalar.copy(dslice, up[:])
                    else:
                        nc.vector.tensor_copy(dslice, up[:])

            def ar_chunk(part, rows, tag):
                """AllReduce one staged s-partial chunk; returns SBUF f16."""
                ar = dram.tile([rows, JO], F16, tag=f"ar{tag}")
                nc.gpsimd.collective_compute(
                    "AllReduce", ALU.add, replica_groups=groups,
                    ins=[part.opt()], outs=[ar.opt()],
                )
                s_c = sqp.tile([rows, JO], F16, tag=f"s{tag}", bufs=1)
                nc.sync.dma_start(out=s_c[:], in_=ar[:])
                return s_c

            def squash(s_sb, first_pass):
                """v16 = squash(alpha*s) over o, alpha = 1/J on the uniform
                first iteration (folded in algebraically: exp-bias ln(J)
                replaces a separate 1/J scaling pass). sqrt(x) is computed as
                exp(0.5*ln(x)) so the whole kernel stays on the ScalarE
                exp+ln activation table set -- Sqrt lives in a different set
                and every switch costs a 1.3us table load on the v critical
                path. s_sb [rows, (o,j)] f16; returns f16 [rows, 512]."""
                rows = s_sb.shape[0]
                a2 = 1.0 / (J * J) if first_pass else 1.0
                sq = sqp.tile([rows, JO], F16, tag="sq")
                nc.vector.tensor_mul(sq[:], s_sb[:], s_sb[:])
                s2 = small.tile([rows, J], F32, tag="sq_s2", bufs=1)
                nc.vector.tensor_reduce(
                    s2[:], sq[:].rearrange("p (o j) -> p j o", j=J), AX.X, ALU.add)
                lns = small.tile([rows, J], F32, tag="sq_lns", bufs=1)
                nc.scalar.activation(lns[:], s2[:], ACTF.Ln,
                                     bias=eps_sb[0:rows, :], scale=a2)
                rt = small.tile([rows, J], F32, tag="sq_rt", bufs=1)
                # rt = exp(0.5*ln(a2*s2+eps) + [ln J]) = J^k * sqrt(a2*s2+eps)
                if first_pass:
                    nc.scalar.activation(rt[:], lns[:], ACTF.Exp,
                                         bias=lnj_sb[0:rows, :], scale=0.5)
                else:
                    nc.scalar.activation(rt[:], lns[:], ACTF.Exp, scale=0.5)
                tmp = small.tile([rows, J], F32, tag="sq_tmp", bufs=1)
                if first_pass:
                    nc.vector.tensor_scalar_mul(tmp[:], s2[:], a2)
                    s2n = tmp
                else:
                    s2n = s2
                opl = small.tile([rows, J], F32, tag="sq_op", bufs=1)
                nc.vector.tensor_scalar_add(opl[:], s2n[:], 1.0)
                den = small.tile([rows, J], F32, tag="sq_den", bufs=1)
                nc.vector.tensor_mul(den[:], opl[:], rt[:])
                rec = small.tile([rows, J], F32, tag="sq_rec", bufs=1)
                nc.vector.reciprocal(rec[:], den[:])
                f = small.tile([rows, J], F32, tag="sq_f", bufs=1)
                nc.vector.tensor_mul(f[:], s2n[:], rec[:])
                v16 = sqp.tile([rows, JO], F16, tag="v16", bufs=2)
                nc.vector.tensor_tensor(
                    v16[:].rearrange("p (o j) -> p o j", j=J),
                    s_sb[:].rearrange("p (o j) -> p o j", j=J),
                    f[:].unsqueeze(1).broadcast_to([rows, O, J]),
                    op=ALU.mult,
                )
                return v16

            def build_vrep(v16, row0, bgs):
                """Replicate v rows (4 bgs of 8 batches) across the 16
                i-groups via PE matmuls with a k=32 selection stationary:
                rep_sb[(g,b8), (bgl,ip,b8')] = (g==bgl)*(b8==b8'), so
                vrep[(ip,b8), bg-cols] = rep[:,bgl].T @ v16."""
                for bgl, bg in enumerate(bgs):
                    vp = upsum.tile([128, JO], F32, tag="u")
                    nc.tensor.matmul(
                        vp[:],
                        lhsT=rep_sb[:, bgl * 128:(bgl + 1) * 128],
                        rhs=v16[:, :],
                        start=True, stop=True,
                    )
                    nc.scalar.copy(vrep[:, bg * JO:(bg + 1) * JO], vp[:])

            # ================= pass A: s0 = (1/32) sum_i u_hat ==============
            s0p = spsum.tile([B, JO], F32, tag="s")
            for it in range(IT):
                nc.tensor.matmul(
                    s0p[:],
                    lhsT=xs_sb[:, it * B:(it + 1) * B],
                    rhs=w_sb[:, it * JO:(it + 1) * JO],
                    start=(it == 0), stop=(it == IT - 1),
                )
            parts_a = []
            for h in (0, 1):
                part = dram.tile([32, JO], F16, tag=f"part{h}")
                st = sqp.tile([32, JO], F16, tag="spst", bufs=1)
                nc.scalar.copy(st[:], s0p[h * 32:(h + 1) * 32, :])
                nc.sync.dma_start(out=part[:], in_=st[:])
                parts_a.append(part)
            s_h0 = ar_chunk(parts_a[0], 32, "h0")
            s_h1 = ar_chunk(parts_a[1], 32, "h1")

            # u16 for bgs 0-3: copies split ScalarE/DVE/GpSimd so the
            # prologue drains fast and all engines are free before the
            # first AllReduce result lands. bgs 4-7 are produced inside
            # pass B (ScalarE copies interleave with its exp/stage slack).
            for bg in range(4):
                produce_u16(bg, "SSSSSSDD")

            # v0 half 0 -> vrep cols 0-3 (h1 deferred into pass B's bg1 so
            # its AR latency never stalls the DVE at the pass boundary)
            build_vrep(squash(s_h0, True), 0, [0, 1, 2, 3])
            carry = (s_h1, True)

            # =================== passes B and C =============================
            def phase_agree(bg, first):
                """uv = u16*v (DVE), o-tree reduce IN PLACE in uv -> logits
                b. Level 1 on DVE; levels 2-4 on GpSimd so the DVE can run
                the PREVIOUS bg's cu concurrently (lag-1 software pipeline).
                DVE and GpSimd never stream the same tile at the same time:
                concurrent same-tile reads measured 3.6x slower."""
                vslice = vrep[:, bg * JO:(bg + 1) * JO]
                bslice = b_state[:, bg * IT * J:(bg + 1) * IT * J]
                u = u16(bg)
                uv = work.tile([128, IT * JO], F16, tag="uv", bufs=2)
                nc.vector.tensor_tensor(
                    uv[:].rearrange("p (t f) -> p t f", f=JO),
                    u[:].rearrange("p (t f) -> p t f", f=JO),
                    vslice.unsqueeze(1).broadcast_to([128, IT, JO]),
                    op=ALU.mult,
                )
                with nc.allow_low_precision(
                        reason="agreement logits are O(1e-2); fp16 keeps "
                               "the DVE adds in 2x packed mode"):
                    uv4 = uv[:].rearrange("p (t o j) -> p t o j", o=O, j=J)
                    nc.vector.tensor_tensor(
                        uv4[:, :, 0:8, :], uv4[:, :, 0:8, :], uv4[:, :, 8:16, :],
                        op=ALU.add)
                    nc.vector.tensor_tensor(
                        uv4[:, :, 0:4, :], uv4[:, :, 0:4, :], uv4[:, :, 4:8, :],
                        op=ALU.add)
                    nc.gpsimd.tensor_tensor(
                        uv4[:, :, 0:2, :], uv4[:, :, 0:2, :], uv4[:, :, 2:4, :],
                        op=ALU.add)
                    dst = (bslice if first else ared_scratch)
                    nc.gpsimd.tensor_tensor(
                        dst.rearrange("p (t j) -> p t j", j=J),
                        uv4[:, :, 0:1, :].rearrange("p t one j -> p (t one) j"),
                        uv4[:, :, 1:2, :].rearrange("p t one j -> p (t one) j"),
                        op=ALU.add)
                    if not first:
                        nc.gpsimd.tensor_add(bslice, bslice, ared_scratch)

            s_state = {}

            def phase_s_a(bg):
                """softmax: exp (ScalarE), esum (DVE reduce + reciprocal),
                then the numerator is normalized IN PLACE by 16 per-i-tile
                ScalarE muls with a per-partition 1/esum scale AP — no DVE
                broadcast-multiply (stride-0-inner fp16xfp32 TT ops measured
                pathologically slow on the DVE)."""
                bslice = b_state[:, bg * IT * J:(bg + 1) * IT * J]
                # logits are O(1e-2): exp without max-subtraction is safe
                ex = work.tile([128, IT * J], F16, tag="ex", bufs=2)
                nc.scalar.activation(ex[:], bslice, ACTF.Exp)
                esum = small.tile([128, IT], F32, tag="esum")
                nc.vector.tensor_reduce(
                    esum[:], ex[:].rearrange("p (t j) -> p t j", j=J),
                    AX.X, ALU.add)
                erec = small.tile([128, IT], F32, tag="erec")
                nc.vector.reciprocal(erec[:], esum[:])
                # rst[(ip,b8), (t, b8')] = (1/esum[b,(t,ip)]) * (b8==b8'):
                # the softmax denominator rides the matmul stationary. Built
                # on GpSimd: broadcast-heavy small TT ops are pathologically
                # slow on the DVE and GpSimd has slack.
                rst = small.tile([128, IT * 8], F16, tag="rst")
                nc.gpsimd.tensor_tensor(
                    rst[:].rearrange("p (t e) -> p t e", e=8),
                    erec[:].unsqueeze(2).broadcast_to([128, IT, 8]),
                    ones_sb[:].unsqueeze(1).broadcast_to([128, IT, 8]),
                    op=ALU.mult,
                )
                s_state[bg] = (ex, rst)

            def phase_s_b(bg, stage_to):
                """softmax-weighted s partial: cu = ex*u16 (one clean-2x DVE
                op), then PSUM matmuls with the 1/esum block-diag stationary
                (softmax denominator folded into the matmul)."""
                u = u16(bg)
                cc, rst = s_state.pop(bg)
                sp = spsum.tile([8, JO], F32, tag="s")
                # cu rotates through the SAME tag as uv: bufs=2 alternation
                # gives each a free slot exactly when its predecessors'
                # readers are done.
                cu = work.tile([128, IT * JO], F16, tag="uv", bufs=2)
                nc.vector.tensor_tensor(
                    cu[:].rearrange("p (t o j) -> p t o j", o=O, j=J),
                    u[:].rearrange("p (t o j) -> p t o j", o=O, j=J),
                    cc[:].rearrange("p (t j) -> p t j", j=J)
                    .unsqueeze(2).broadcast_to([128, IT, O, J]),
                    op=ALU.mult,
                )
                for t in range(IT):
                    nc.tensor.matmul(
                        sp[:],
                        lhsT=rst[:, t * 8:(t + 1) * 8],
                        rhs=cu[:, t * JO:(t + 1) * JO],
                        start=(t == 0), stop=(t == IT - 1),
                    )
                part, row0 = stage_to
                st = sqp.tile([8, JO], F16, tag="spst", bufs=1)
                nc.scalar.copy(st[:], sp[:])
                nc.sync.dma_start(out=part[row0:row0 + 8, :], in_=st[:])

            # ---------------- pass B (iteration 1) --------------------------
            # Lag-1 software pipeline: phase_s(bg) runs one slot behind
            # phase_agree(bg+1), so the DVE streams cu(bg) while GpSimd
            # chews bg+1's tree and ScalarE computes bg+1's exp.
            partB0 = dram.tile([32, JO], F16, tag="partB0")
            partB1 = dram.tile([32, JO], F16, tag="partB1")
            late_prod = [4, 5, 7, 6]
            for k in range(NBG + 2):
                if k >= 2:
                    bg = k - 2
                    pt, r0 = (partB0, bg * 8) if bg < 4 else (partB1, (bg - 4) * 8)
                    phase_s_b(bg, (pt, r0))
                    if bg < 4:
                        # late u16 production rides pass B's PE/ScalarE slack
                        produce_u16(late_prod[bg], "SSSSSSSS")
                    if bg == 3:
                        s_bh0 = ar_chunk(partB0, 32, "h0")
                    if bg == 5:
                        # v1 half 0 -> vrep cols 0-3 for pass C
                        build_vrep(squash(s_bh0, False), 0, [0, 1, 2, 3])
                if k < NBG:
                    phase_agree(k, True)
                    if k == 1 and carry is not None:
                        cs, cfirst = carry
                        build_vrep(squash(cs, cfirst), 32, [4, 5, 6, 7])
                        carry = None
                if 1 <= k <= NBG:
                    phase_s_a(k - 1)
            s_bh1 = ar_chunk(partB1, 32, "h1")
            carry = (s_bh1, False)

            # ---------------- pass C (iteration 2) --------------------------
            partCb = dram.tile([48, JO], F16, tag="partCb")
            partC6 = dram.tile([8, JO], F16, tag="partC6")
            partC7 = dram.tile([8, JO], F16, tag="partC7")
            for k in range(NBG + 2):
                if k >= 2:
                    bg = k - 2
                    stage = ((partCb, bg * 8) if bg < 6 else
                             (partC6, 0) if bg == 6 else (partC7, 0))
                    phase_s_b(bg, stage)
                    if bg == 5:
                        s_cb = ar_chunk(partCb, 48, "cb")
                    if bg == 6:
                        v_cb = squash(s_cb, False)
                        nc.sync.dma_start(out=v_out[0:48, :], in_=v_cb[:])
                        s_c6 = ar_chunk(partC6, 8, "c6")
                    if bg == 7:
                        v_c6 = squash(s_c6, False)
                        nc.sync.dma_start(out=v_out[48:56, :], in_=v_c6[:])
                if k < NBG:
                    phase_agree(k, False)
                    if k == 1 and carry is not None:
                        cs, cfirst = carry
                        build_vrep(squash(cs, cfirst), 32, [4, 5, 6, 7])
                        carry = None
                if 1 <= k <= NBG:
                    phase_s_a(k - 1)
            s_c7 = ar_chunk(partC7, 8, "c7")
            v_c7 = squash(s_c7, False)
            nc.sync.dma_start(out=v_out[56:64, :], in_=v_c7[:])
    _split_multi_waits(nc)
    return nc


def prep_inputs(x, W):
    """Host-side layout prep. x [64,2048,8] f32, W [1,2048,32,16,8] f32."""
    x = np.ascontiguousarray(x, dtype=np.float32).astype(np.float16)
    Wf = np.ascontiguousarray(W, dtype=np.float32)[0].astype(np.float16)
    in_maps = []
    ones_bd = np.tile(np.eye(8, dtype=np.float16), (16, 1))  # [(ip,b8), 8]
    # rep[(g,b8), (bgl,ip,b8')] = (g==bgl)*(b8==b8'): v-replication selector
    rep_bd = np.zeros((4, 8, 4, 16, 8), dtype=np.float16)
    for g in range(4):
        for b in range(8):
            rep_bd[g, b, g, :, b] = 1.0
    rep_bd = rep_bd.reshape(32, 512)
    for c in range(N_CORES):
        i0 = c * IL
        Wl = Wf[i0:i0 + IL].reshape(IT, 16, J, O, D)         # [it, i16, j, o, d]
        w_in = np.ascontiguousarray(
            Wl.transpose(1, 4, 0, 3, 2)).reshape(128, IT * JO)  # (i16,d),(it,o,j)
        xl = x[:, i0:i0 + IL, :].reshape(B, IT, 16, D)        # [b, it, i16, d]
        xt = np.ascontiguousarray(xl.transpose(2, 3, 1, 0))   # [i16, d, it, b]
        xs_in = xt.reshape(128, IT * B)
        # block-diag x, bg-major: [i16, d, bg, it, ip, b8], nonzero ip == i16
        bdx = np.zeros((16, D, NBG, IT, 16, 8), dtype=np.float16)
        xg = xt.reshape(16, D, IT, NBG, 8).transpose(0, 1, 3, 2, 4)
        idx = np.arange(16)
        bdx[idx, :, :, :, idx, :] = xg[idx]
        in_maps.append({
            "w_in": w_in,
            "xs_in": xs_in,
            "bdx_in": bdx.reshape(128, IT * NBG * 128),
            "ones_in": ones_bd,
            "rep_in": rep_bd,
        })
    return in_maps


def postprocess(v_raw):
    """Device v_out is [B, (o,j)] f16; return [B, J, O] f32."""
    return np.ascontiguousarray(
        np.asarray(v_raw).astype(np.float32).reshape(B, O, J).transpose(0, 2, 1))


def kernel(x, W):
    nc = build_nc()
    in_maps = prep_inputs(np.asarray(x), np.asarray(W))
    res = run_bass_kernel_spmd(nc, in_maps, core_ids=list(range(N_CORES)))
    return postprocess(res.results[0]["v_out"])


if __name__ == "__main__":
    rng = np.random.default_rng(0)
    x = rng.standard_normal((B, I, D), dtype=np.float32)
    W = (0.01 * rng.standard_normal((1, I, J, O, D))).astype(np.float32)
    v = kernel(x, W)
    print("kernel output", v.shape, v.dtype, float(np.abs(v).max()))
